# revision 15
# baseline (speedup 1.0000x reference)
"""Multi-head attention (no qkv proj) + out_proj, sharded over 8 TRN2 cores.

Sharding: core i handles batch b = i//4, query rows tc = (i//2)%2 of 512,
and head group hg = i%2 (8 of 16 heads).  out_proj weight is column-sharded
over the head groups; the "all-reduce" is a host-side partial-sum of the two
head-group outputs (plus out_b) at gather time.

v3 pipeline: heads processed in pairs (2c2, 2c2+1) sharing the kT/qT chunk
c2.  Per s-chunk, the two heads' score matmuls run CONCURRENTLY on disjoint
PE row-groups (auto tile_position from base_partition 0/64), writing the two
banks of one [128, 1024] PSUM tile.  One ACT exp and one contiguous fp16 DVE
multiply (2x rate) then cover both heads.  exp(bias) is host-precomputed
fp16 with the key-padding mask baked in as exact zeros, laid out
[quad][s][4h][t] so DMA lines are 4 KB.  All DMAs ride the single sync-HWDGE
FIFO in pipeline order (the ring is FIFO, so order = priority); the first
quad's bias is split into 4x1MB pieces so the pipeline starts early.
Softmax denominators ride as a 65th row of the V matmul; 1/den is broadcast
across partitions via a K=1 PE matmul.  out_proj bias is added on the host
during the gather.
"""

import numpy as np

import concourse.mybir as mybir
import concourse.tile as tile
from concourse import bacc
from concourse.bass_utils import run_bass_kernel_spmd

F32 = mybir.dt.float32
F16 = mybir.dt.float16
NP16 = np.float16

P = 128          # partitions
T = 512          # query rows per core
S = 1024         # key length
H = 8            # heads per core (of 16)
HD = 64          # head dim
DIN = H * HD     # local d_model slice (512)
NDIN = DIN // P  # 4 chunks = 4 head pairs
DM = 1024        # full d_model
NS = S // P      # 8 s-chunks
ND = DM // P     # 8 d_out chunks
SCALE = HD ** -0.5
EXP_SHIFT = -2.0  # exp(x-2): keeps fp16 exp outputs well inside range

AF = mybir.ActivationFunctionType


def build_bass():
    nc = bacc.Bacc()

    qT_d = nc.dram_tensor("qT", [DIN, T], F16, kind="ExternalInput")
    kT_d = nc.dram_tensor("kT", [DIN, S], F16, kind="ExternalInput")
    vaug_d = nc.dram_tensor("vaug", [S, H * (HD + 1)], F16, kind="ExternalInput")
    # exp(bias) per head quad, [quad][s][4h][t] so DMA lines are 4 KB
    biasQ_d = nc.dram_tensor("biasQ", [2, S, 4, T], F16, kind="ExternalInput")
    wT_d = nc.dram_tensor("wT", [DIN, DM], F16, kind="ExternalInput")
    outT_d = nc.dram_tensor("outT", [T, DM], F16, kind="ExternalOutput")

    VW = HD + 1  # columns per head in vaug
    QW = 4 * T   # bias columns per s-row in a quad

    with tile.TileContext(nc) as tc, nc.allow_low_precision(reason="fp16 matmul pipeline"):
        with (
            tc.tile_pool(name="weights", bufs=1) as wpool,
            tc.tile_pool(name="bias", bufs=2) as bpool,
            tc.tile_pool(name="expv", bufs=2) as epool,
            tc.tile_pool(name="small", bufs=4) as spool,
            tc.tile_pool(name="osb", bufs=1) as opool_sb,
        ):
            qT_t = [wpool.tile([P, T], F16, name=f"qT{c}", tag=f"qT{c}") for c in range(NDIN)]
            kT_t = [wpool.tile([P, S], F16, name=f"kT{c}", tag=f"kT{c}") for c in range(NDIN)]
            vaug_t = wpool.tile([P, NS * H * VW], F16, name="vaug", tag="vaug")
            wT_t = [wpool.tile([P, DM], F16, name=f"wT{c}", tag=f"wT{c}") for c in range(NDIN)]
            eshift_t = wpool.tile([P, 1], F32, name="eshift", tag="eshift")
            nc.vector.memset(eshift_t[:], EXP_SHIFT)
            warm_t = wpool.tile([P, 512], F16, name="warm", tag="warm")
            nc.vector.memset(warm_t[:], 0.0)
            ones_t = wpool.tile([P, HD], F16, name="ones", tag="ones")
            nc.vector.memset(ones_t[:], 1.0)
            aflat_t = [wpool.tile([P, T], F16, name=f"af{c}", tag=f"af{c}") for c in range(NDIN)]

            # DMA FIFO order = pipeline order
            nc.sync.dma_start(out=kT_t[0][:], in_=kT_d[0:P, :])
            nc.sync.dma_start(out=qT_t[0][:], in_=qT_d[0:P, :])

            bias_q = [None, None]

            def bias_dma(qi, sc0, sc1):
                """DMA s-chunks [sc0, sc1) of bias quad qi."""
                nc.sync.dma_start(
                    out=bias_q[qi][:, sc0 * QW:sc1 * QW]
                        .rearrange("p (sc x) -> p sc x", sc=sc1 - sc0),
                    in_=biasQ_d[qi, sc0 * P:sc1 * P]
                        .rearrange("(sc p) h4 t -> p sc (h4 t)", p=P),
                )

            bias_q[0] = bpool.tile([P, NS * QW], F16, name="biasq", tag="biasq")
            bias_dma(0, 0, 1)    # 0.5MB: lets the first exp start early
            bias_dma(0, 1, 3)    # 1MB

            def pair_step(p, expv_prev, avs_prev, scps, avps):
                """scores+exp+mul for pair p, interleaved chunk-by-chunk with
                the AV accumulation matmuls of pair p-1 so the PE instruction
                stream stays dense (HAM stays unthrottled)."""
                q, pq = divmod(p, 2)
                bias_sb = bias_q[q] if p >= 0 else None
                expv = None
                if p >= 0:
                    expv = epool.tile([P, NS * 2 * T], F16, name="expv", tag="expv")
                loA = slice(0, HD)
                loB = slice(HD, P)
                for sc in range(NS):
                    if p >= 0:
                        sc_ps = scps.tile([P, 2 * T], F32, name="sc", tag="sc")
                        nc.tensor.matmul(
                            sc_ps[:, 0:T],
                            kT_t[p][loA, sc * P:(sc + 1) * P],
                            qT_t[p][loA, :],
                            start=True, stop=True,
                        )
                        nc.tensor.matmul(
                            sc_ps[:, T:2 * T],
                            kT_t[p][loB, sc * P:(sc + 1) * P],
                            qT_t[p][loB, :],
                            start=True, stop=True,
                        )
                    if avs_prev is not None:
                        for h2 in range(2):
                            lh = 2 * (p - 1 if p >= 0 else NDIN - 1) + h2
                            nc.tensor.matmul(
                                avs_prev[h2][:],
                                vaug_t[:, sc * H * VW + lh * VW: sc * H * VW + (lh + 1) * VW],
                                expv_prev[:, sc * 2 * T + h2 * T: sc * 2 * T + (h2 + 1) * T],
                                start=(sc == 0), stop=(sc == NS - 1),
                                skip_group_check=True,
                            )
                    if p >= 0:
                        sl = slice(sc * 2 * T, (sc + 1) * 2 * T)
                        nc.scalar.activation(
                            expv[:, sl], sc_ps[:], AF.Exp,
                            bias=eshift_t[:], scale=SCALE,
                        )
                        nc.vector.tensor_mul(
                            expv[:, sl], expv[:, sl],
                            bias_sb[:, sc * QW + pq * 2 * T: sc * QW + (pq + 1) * 2 * T],
                        )
                return expv

            def divide_pair(p, avs, scps):
                """1/den broadcast + aflat writes for pair p (avs = [av_A, av_B]).
                Both broadcasts col-tile into one scores-pool bank."""
                rcps = []
                for h2 in range(2):
                    av_ps = avs[h2]
                    den_sb = spool.tile([1, T], F32, name="den", tag="den")
                    nc.vector.tensor_copy(den_sb[:], av_ps[HD:HD + 1, :])
                    rcp = spool.tile([1, T], F32, name="rcp", tag="rcp")
                    nc.vector.reciprocal_approx_fast(rcp[:], den_sb[:])
                    rcp16 = spool.tile([1, T], F16, name="rcp16", tag="rcp16")
                    nc.vector.tensor_copy(rcp16[:], rcp[:])
                    rcps.append(rcp16)
                bc_ps = scps.tile([P, 2 * T], F32, name="sc", tag="sc")
                for h2 in range(2):
                    nc.tensor.matmul(
                        bc_ps[h2 * HD:(h2 + 1) * HD, 0:T], ones_t[0:1, :],
                        rcps[h2][:], start=True, stop=True,
                    )
                for h2 in range(2):
                    bc_sb = spool.tile([HD, T], F32, name="bc", tag="bc")
                    nc.vector.tensor_copy(bc_sb[:], bc_ps[h2 * HD:(h2 + 1) * HD, 0:T])
                    nc.vector.tensor_mul(
                        aflat_t[p][h2 * HD:(h2 + 1) * HD, :],
                        avs[h2][0:HD, :], bc_sb[:],
                    )

            with (
                tc.tile_pool(name="scps", bufs=3, space="PSUM") as scps,
                tc.tile_pool(name="avps", bufs=2, space="PSUM") as avps,
            ):
                wm_ps = scps.tile([P, 512], F32, name="wm", tag="sc")
                for _ in range(2):
                    nc.tensor.matmul(wm_ps[:], warm_t[:, 0:P], warm_t[:],
                                     start=True, stop=True)

                expv_prev = None
                avs_prev = None
                for p in range(NDIN):
                    if p == 0:
                        c = 1
                        nc.sync.dma_start(out=kT_t[c][:], in_=kT_d[c * P:(c + 1) * P, :])
                        nc.sync.dma_start(out=qT_t[c][:], in_=qT_d[c * P:(c + 1) * P, :])
                        bias_dma(0, 3, 5)   # 1MB
                        bias_dma(0, 5, 7)   # 1MB
                        bias_dma(0, 7, 8)   # 0.5MB
                        nc.sync.dma_start(
                            out=vaug_t[:].rearrange("p (sc x) -> p sc x", sc=NS),
                            in_=vaug_d[:, :].rearrange("(sc p) x -> p sc x", p=P),
                        )
                        c = 2
                        nc.sync.dma_start(out=kT_t[c][:], in_=kT_d[c * P:(c + 1) * P, :])
                        nc.sync.dma_start(out=qT_t[c][:], in_=qT_d[c * P:(c + 1) * P, :])
                        bias_q[1] = bpool.tile([P, NS * QW], F16, name="biasq", tag="biasq")
                        bias_dma(1, 0, 4)   # 2MB
                        c = 3
                        nc.sync.dma_start(out=kT_t[c][:], in_=kT_d[c * P:(c + 1) * P, :])
                        nc.sync.dma_start(out=qT_t[c][:], in_=qT_d[c * P:(c + 1) * P, :])
                        bias_dma(1, 4, 8)   # 2MB
                    if p == 1:
                        for c in range(NDIN):
                            nc.sync.dma_start(out=wT_t[c][:], in_=wT_d[c * P:(c + 1) * P, :])

                    avs = None
                    if p > 0:
                        avs = [avps.tile([HD + 1, T], F32, name="av", tag="av")
                               for _ in range(2)]
                    expv = pair_step(p, expv_prev, avs, scps, avps)
                    if p > 0:
                        divide_pair(p - 1, avs, scps)
                    expv_prev = expv
                avs = [avps.tile([HD + 1, T], F32, name="av", tag="av")
                       for _ in range(2)]
                pair_step(-1, expv_prev, avs, scps, avps)
                divide_pair(NDIN - 1, avs, scps)

            # ---- out_proj: out2[t, dout] = attnflat^T-chunk @ W ----
            # aflat chunk is the stationary operand: 2 matmuls (dout halves)
            # per LDWEIGHTS instead of a reload per matmul.
            osb = opool_sb.tile([P, 4 * DM], F16, name="osb", tag="osb")
            NT = T // P  # 4 t-chunks
            with tc.tile_pool(name="ops", bufs=2, space="PSUM") as ops:
                for tc_c in range(NT):
                    o_ps = ops.tile([P, DM], F32, name="o", tag="o")
                    for dinc in range(NDIN):
                        for dh in range(2):
                            nc.tensor.matmul(
                                o_ps[:, dh * T:(dh + 1) * T],
                                aflat_t[dinc][:, tc_c * P:(tc_c + 1) * P],
                                wT_t[dinc][:, dh * T:(dh + 1) * T],
                                start=(dinc == 0), stop=(dinc == NDIN - 1),
                            )
                    osl = slice(tc_c * DM, (tc_c + 1) * DM)
                    nc.any.tensor_copy(osb[:, osl], o_ps[:])
                    nc.sync.dma_start(
                        out=outT_d[tc_c * P:(tc_c + 1) * P, :],
                        in_=osb[:, osl],
                    )

    nc.finalize()
    return nc


_NC = None


def _get_nc():
    global _NC
    if _NC is None:
        _NC = build_bass()
    return _NC


def _core_index(b, tc_i, hg):
    return b * 4 + tc_i * 2 + hg


def _make_in_maps(query, key, value, attn_bias, key_padding_mask, out_w, out_b):
    query = np.asarray(query, dtype=np.float32)
    key = np.asarray(key, dtype=np.float32)
    value = np.asarray(value, dtype=np.float32)
    attn_bias = np.asarray(attn_bias, dtype=np.float32)
    mask = np.asarray(key_padding_mask).astype(bool)
    out_w = np.asarray(out_w, dtype=np.float32)

    wT_full = np.ascontiguousarray(out_w.T).astype(NP16)   # [din, dout]

    maps = [None] * 8
    for b in range(2):
        kT_full = np.ascontiguousarray(key[b].T).astype(NP16)  # [1024, 1024]
        for hg in range(2):
            hs = hg * H              # first global head of the group
            ds = hg * DIN            # first d_model row of the group
            vaug = np.ones((S, H * (HD + 1)), NP16)
            vaug.reshape(S, H, HD + 1)[:, :, :HD] = (
                value[b, :, ds:ds + DIN].reshape(S, H, HD))
            kT = np.ascontiguousarray(kT_full[ds:ds + DIN])
            wT = np.ascontiguousarray(wT_full[ds:ds + DIN])
            for tc_i in range(2):
                t0 = tc_i * T
                # [quad, s, 4h, t] layout, exp() with mask rows zeroed
                biasQ = np.ascontiguousarray(
                    attn_bias[b, hs:hs + H, t0:t0 + T, :]
                    .reshape(2, 4, T, S).transpose(0, 3, 1, 2))
                biasQ[:, mask[b], :, :] = -10000.0
                np.exp(biasQ, out=biasQ)
                qT = np.ascontiguousarray(
                    query[b, t0:t0 + T, ds:ds + DIN].T).astype(NP16)
                maps[_core_index(b, tc_i, hg)] = {
                    "qT": qT, "kT": kT, "vaug": vaug,
                    "biasQ": biasQ.astype(NP16), "wT": wT,
                }
    return maps


def run(inputs, trace=False, **run_kwargs):
    """Returns (output [2,1024,1024] f32, BassKernelResults)."""
    nc = _get_nc()
    in_maps = _make_in_maps(**inputs)
    res = run_bass_kernel_spmd(
        nc, in_maps, core_ids=list(range(8)), trace=trace, **run_kwargs
    )
    out_b = np.asarray(inputs["out_b"], dtype=np.float32)
    out = np.empty((2, S, DM), np.float32)
    for b in range(2):
        for tc_i in range(2):
            part = (np.asarray(res.results[_core_index(b, tc_i, 0)]["outT"], dtype=np.float32)
                    + np.asarray(res.results[_core_index(b, tc_i, 1)]["outT"], dtype=np.float32))
            out[b, tc_i * T:(tc_i + 1) * T, :] = part + out_b[None, :]
    return out, res


def kernel(**inputs):
    out, _ = run(inputs, trace=False)
    return out


# revision 16
# speedup vs baseline: 1.1563x; 1.1563x over previous
"""Multi-head attention (no qkv proj) + out_proj, sharded over 8 TRN2 cores.

Sharding: core i handles batch b = i//4, query rows tc = (i//2)%2 of 512,
and head group hg = i%2 (8 of 16 heads).  out_proj weight is column-sharded
over the head groups; the "all-reduce" is a host-side partial-sum of the two
head-group outputs (plus out_b) at gather time.

v3 pipeline: heads processed in pairs (2c2, 2c2+1) sharing the kT/qT chunk
c2.  Per s-chunk, the two heads' score matmuls run CONCURRENTLY on disjoint
PE row-groups (auto tile_position from base_partition 0/64), writing the two
banks of one [128, 1024] PSUM tile.  One ACT exp and one contiguous fp16 DVE
multiply (2x rate) then cover both heads.  exp(bias) is host-precomputed
fp16 with the key-padding mask baked in as exact zeros, laid out
[quad][s][4h][t] so DMA lines are 4 KB.  All DMAs ride the single sync-HWDGE
FIFO in pipeline order (the ring is FIFO, so order = priority); the first
quad's bias is split into 4x1MB pieces so the pipeline starts early.
Softmax denominators ride as a 65th row of the V matmul; 1/den is broadcast
across partitions via a K=1 PE matmul.  out_proj bias is added on the host
during the gather.
"""

import numpy as np

import concourse.mybir as mybir
import concourse.tile as tile
from concourse import bacc
from concourse.bass_utils import run_bass_kernel_spmd

F32 = mybir.dt.float32
F16 = mybir.dt.float16
NP16 = np.float16

P = 128          # partitions
T = 512          # query rows per core
S = 1024         # key length
H = 8            # heads per core (of 16)
HD = 64          # head dim
DIN = H * HD     # local d_model slice (512)
NDIN = DIN // P  # 4 chunks = 4 head pairs
DM = 1024        # full d_model
NS = S // P      # 8 s-chunks
ND = DM // P     # 8 d_out chunks
SCALE = HD ** -0.5
EXP_SHIFT = -2.0  # exp(x-2): keeps fp16 exp outputs well inside range

AF = mybir.ActivationFunctionType


def build_bass():
    nc = bacc.Bacc()

    qT_d = nc.dram_tensor("qT", [DIN, T], F16, kind="ExternalInput")
    kT_d = nc.dram_tensor("kT", [DIN, S], F16, kind="ExternalInput")
    vaug_d = nc.dram_tensor("vaug", [S, H * (HD + 1)], F16, kind="ExternalInput")
    # exp(bias) per head quad, [quad][s][4h][t] so DMA lines are 4 KB
    biasQ_d = nc.dram_tensor("biasQ", [2, S, 4, T], F16, kind="ExternalInput")
    wT_d = nc.dram_tensor("wT", [DIN, DM], F16, kind="ExternalInput")
    outT_d = nc.dram_tensor("outT", [T, DM], F16, kind="ExternalOutput")

    VW = HD + 1  # columns per head in vaug
    QW = 4 * T   # bias columns per s-row in a quad

    with tile.TileContext(nc) as tc, nc.allow_low_precision(reason="fp16 matmul pipeline"):
        with (
            tc.tile_pool(name="weights", bufs=1) as wpool,
            tc.tile_pool(name="bias", bufs=2) as bpool,
            tc.tile_pool(name="expv", bufs=2) as epool,
            tc.tile_pool(name="small", bufs=4) as spool,
            tc.tile_pool(name="osb", bufs=1) as opool_sb,
        ):
            qT_t = [wpool.tile([P, T], F16, name=f"qT{c}", tag=f"qT{c}") for c in range(NDIN)]
            kT_t = [wpool.tile([P, S], F16, name=f"kT{c}", tag=f"kT{c}") for c in range(NDIN)]
            vaug_t = wpool.tile([P, NS * H * VW], F16, name="vaug", tag="vaug")
            wT_t = [wpool.tile([P, DM], F16, name=f"wT{c}", tag=f"wT{c}") for c in range(NDIN)]
            eshift_t = wpool.tile([P, 1], F32, name="eshift", tag="eshift")
            nc.vector.memset(eshift_t[:], EXP_SHIFT)
            warm_t = wpool.tile([P, 512], F16, name="warm", tag="warm")
            nc.vector.memset(warm_t[:], 0.0)
            ones_t = wpool.tile([P, HD], F16, name="ones", tag="ones")
            nc.vector.memset(ones_t[:], 1.0)
            aflat_t = [wpool.tile([P, T], F16, name=f"af{c}", tag=f"af{c}") for c in range(NDIN)]

            # DMA FIFO order = pipeline order
            nc.sync.dma_start(out=kT_t[0][:], in_=kT_d[0:P, :])
            nc.sync.dma_start(out=qT_t[0][:], in_=qT_d[0:P, :])

            bias_q = [None, None]

            def bias_dma(qi, sc0, sc1):
                """DMA s-chunks [sc0, sc1) of bias quad qi."""
                nc.sync.dma_start(
                    out=bias_q[qi][:, sc0 * QW:sc1 * QW]
                        .rearrange("p (sc x) -> p sc x", sc=sc1 - sc0),
                    in_=biasQ_d[qi, sc0 * P:sc1 * P]
                        .rearrange("(sc p) h4 t -> p sc (h4 t)", p=P),
                )

            bias_q[0] = bpool.tile([P, NS * QW], F16, name="biasq", tag="biasq")
            bias_dma(0, 0, 1)    # 0.5MB: lets the first exp start early
            bias_dma(0, 1, 3)    # 1MB

            def pair_step(p, expv_prev, avs_prev, scps, avps):
                """scores+exp+mul for pair p, interleaved chunk-by-chunk with
                the AV accumulation matmuls of pair p-1 so the PE instruction
                stream stays dense (HAM stays unthrottled)."""
                q, pq = divmod(p, 2)
                bias_sb = bias_q[q] if p >= 0 else None
                expv = None
                if p >= 0:
                    expv = epool.tile([P, NS * 2 * T], F16, name="expv", tag="expv")
                loA = slice(0, HD)
                loB = slice(HD, P)
                for sc in range(NS):
                    if p >= 0:
                        sc_ps = scps.tile([P, 2 * T], F32, name="sc", tag="sc")
                        nc.tensor.matmul(
                            sc_ps[:, 0:T],
                            kT_t[p][loA, sc * P:(sc + 1) * P],
                            qT_t[p][loA, :],
                            start=True, stop=True,
                        )
                        nc.tensor.matmul(
                            sc_ps[:, T:2 * T],
                            kT_t[p][loB, sc * P:(sc + 1) * P],
                            qT_t[p][loB, :],
                            start=True, stop=True,
                        )
                    if avs_prev is not None:
                        for h2 in range(2):
                            lh = 2 * (p - 1 if p >= 0 else NDIN - 1) + h2
                            nc.tensor.matmul(
                                avs_prev[h2][:],
                                vaug_t[:, sc * H * VW + lh * VW: sc * H * VW + (lh + 1) * VW],
                                expv_prev[:, sc * 2 * T + h2 * T: sc * 2 * T + (h2 + 1) * T],
                                start=(sc == 0), stop=(sc == NS - 1),
                                skip_group_check=True,
                            )
                    if p >= 0:
                        sl = slice(sc * 2 * T, (sc + 1) * 2 * T)
                        nc.scalar.activation(
                            expv[:, sl], sc_ps[:], AF.Exp,
                            bias=eshift_t[:], scale=SCALE,
                        )
                        nc.vector.tensor_mul(
                            expv[:, sl], expv[:, sl],
                            bias_sb[:, sc * QW + pq * 2 * T: sc * QW + (pq + 1) * 2 * T],
                        )
                return expv

            def divide_pair(p, avs, scps):
                """1/den broadcast + aflat writes for pair p (avs = [av_A, av_B]).
                Both broadcasts col-tile into one scores-pool bank."""
                rcps = []
                for h2 in range(2):
                    av_ps = avs[h2]
                    den_sb = spool.tile([1, T], F32, name="den", tag="den")
                    nc.vector.tensor_copy(den_sb[:], av_ps[HD:HD + 1, :])
                    rcp = spool.tile([1, T], F32, name="rcp", tag="rcp")
                    nc.vector.reciprocal_approx_fast(rcp[:], den_sb[:])
                    rcp16 = spool.tile([1, T], F16, name="rcp16", tag="rcp16")
                    nc.vector.tensor_copy(rcp16[:], rcp[:])
                    rcps.append(rcp16)
                bc_ps = scps.tile([P, 2 * T], F32, name="sc", tag="sc")
                for h2 in range(2):
                    nc.tensor.matmul(
                        bc_ps[h2 * HD:(h2 + 1) * HD, 0:T], ones_t[0:1, :],
                        rcps[h2][:], start=True, stop=True,
                    )
                for h2 in range(2):
                    bc_sb = spool.tile([HD, T], F32, name="bc", tag="bc")
                    nc.vector.tensor_copy(bc_sb[:], bc_ps[h2 * HD:(h2 + 1) * HD, 0:T])
                    nc.vector.tensor_mul(
                        aflat_t[p][h2 * HD:(h2 + 1) * HD, :],
                        avs[h2][0:HD, :], bc_sb[:],
                    )

            with (
                tc.tile_pool(name="scps", bufs=2, space="PSUM") as scps,
                tc.tile_pool(name="avps", bufs=4, space="PSUM") as avps,
            ):
                wm_ps = scps.tile([P, 512], F32, name="wm", tag="sc")
                for _ in range(2):
                    nc.tensor.matmul(wm_ps[:], warm_t[:, 0:P], warm_t[:],
                                     start=True, stop=True)

                expv_prev = None
                avs_prev = None
                for p in range(NDIN):
                    if p == 0:
                        c = 1
                        nc.sync.dma_start(out=kT_t[c][:], in_=kT_d[c * P:(c + 1) * P, :])
                        nc.sync.dma_start(out=qT_t[c][:], in_=qT_d[c * P:(c + 1) * P, :])
                        bias_dma(0, 3, 5)   # 1MB
                        bias_dma(0, 5, 7)   # 1MB
                        bias_dma(0, 7, 8)   # 0.5MB
                        nc.sync.dma_start(
                            out=vaug_t[:].rearrange("p (sc x) -> p sc x", sc=NS),
                            in_=vaug_d[:, :].rearrange("(sc p) x -> p sc x", p=P),
                        )
                        c = 2
                        nc.sync.dma_start(out=kT_t[c][:], in_=kT_d[c * P:(c + 1) * P, :])
                        nc.sync.dma_start(out=qT_t[c][:], in_=qT_d[c * P:(c + 1) * P, :])
                        bias_q[1] = bpool.tile([P, NS * QW], F16, name="biasq", tag="biasq")
                        bias_dma(1, 0, 4)   # 2MB
                        c = 3
                        nc.sync.dma_start(out=kT_t[c][:], in_=kT_d[c * P:(c + 1) * P, :])
                        nc.sync.dma_start(out=qT_t[c][:], in_=qT_d[c * P:(c + 1) * P, :])
                        bias_dma(1, 4, 8)   # 2MB
                    if p == 1:
                        for c in range(NDIN):
                            nc.sync.dma_start(out=wT_t[c][:], in_=wT_d[c * P:(c + 1) * P, :])

                    avs = None
                    if p > 0:
                        avs = [avps.tile([HD + 1, T], F32, name="av", tag="av")
                               for _ in range(2)]
                    expv = pair_step(p, expv_prev, avs, scps, avps)
                    if p > 0:
                        divide_pair(p - 1, avs, scps)
                    expv_prev = expv
                avs = [avps.tile([HD + 1, T], F32, name="av", tag="av")
                       for _ in range(2)]
                pair_step(-1, expv_prev, avs, scps, avps)
                divide_pair(NDIN - 1, avs, scps)

            # ---- out_proj: out2[t, dout] = attnflat^T-chunk @ W ----
            # aflat chunk is the stationary operand: 2 matmuls (dout halves)
            # per LDWEIGHTS instead of a reload per matmul.
            osb = opool_sb.tile([P, 4 * DM], F16, name="osb", tag="osb")
            NT = T // P  # 4 t-chunks
            with tc.tile_pool(name="ops", bufs=2, space="PSUM") as ops:
                for tc_c in range(NT):
                    o_ps = ops.tile([P, DM], F32, name="o", tag="o")
                    for dinc in range(NDIN):
                        for dh in range(2):
                            nc.tensor.matmul(
                                o_ps[:, dh * T:(dh + 1) * T],
                                aflat_t[dinc][:, tc_c * P:(tc_c + 1) * P],
                                wT_t[dinc][:, dh * T:(dh + 1) * T],
                                start=(dinc == 0), stop=(dinc == NDIN - 1),
                            )
                    osl = slice(tc_c * DM, (tc_c + 1) * DM)
                    nc.any.tensor_copy(osb[:, osl], o_ps[:])
                    nc.sync.dma_start(
                        out=outT_d[tc_c * P:(tc_c + 1) * P, :],
                        in_=osb[:, osl],
                    )

    nc.finalize()
    return nc


_NC = None


def _get_nc():
    global _NC
    if _NC is None:
        _NC = build_bass()
    return _NC


def _core_index(b, tc_i, hg):
    return b * 4 + tc_i * 2 + hg


def _make_in_maps(query, key, value, attn_bias, key_padding_mask, out_w, out_b):
    query = np.asarray(query, dtype=np.float32)
    key = np.asarray(key, dtype=np.float32)
    value = np.asarray(value, dtype=np.float32)
    attn_bias = np.asarray(attn_bias, dtype=np.float32)
    mask = np.asarray(key_padding_mask).astype(bool)
    out_w = np.asarray(out_w, dtype=np.float32)

    wT_full = np.ascontiguousarray(out_w.T).astype(NP16)   # [din, dout]

    maps = [None] * 8
    for b in range(2):
        kT_full = np.ascontiguousarray(key[b].T).astype(NP16)  # [1024, 1024]
        for hg in range(2):
            hs = hg * H              # first global head of the group
            ds = hg * DIN            # first d_model row of the group
            vaug = np.ones((S, H * (HD + 1)), NP16)
            vaug.reshape(S, H, HD + 1)[:, :, :HD] = (
                value[b, :, ds:ds + DIN].reshape(S, H, HD))
            kT = np.ascontiguousarray(kT_full[ds:ds + DIN])
            wT = np.ascontiguousarray(wT_full[ds:ds + DIN])
            for tc_i in range(2):
                t0 = tc_i * T
                # [quad, s, 4h, t] layout, exp() with mask rows zeroed
                biasQ = np.ascontiguousarray(
                    attn_bias[b, hs:hs + H, t0:t0 + T, :]
                    .reshape(2, 4, T, S).transpose(0, 3, 1, 2))
                biasQ[:, mask[b], :, :] = -10000.0
                np.exp(biasQ, out=biasQ)
                qT = np.ascontiguousarray(
                    query[b, t0:t0 + T, ds:ds + DIN].T).astype(NP16)
                maps[_core_index(b, tc_i, hg)] = {
                    "qT": qT, "kT": kT, "vaug": vaug,
                    "biasQ": biasQ.astype(NP16), "wT": wT,
                }
    return maps


def run(inputs, trace=False, **run_kwargs):
    """Returns (output [2,1024,1024] f32, BassKernelResults)."""
    nc = _get_nc()
    in_maps = _make_in_maps(**inputs)
    res = run_bass_kernel_spmd(
        nc, in_maps, core_ids=list(range(8)), trace=trace, **run_kwargs
    )
    out_b = np.asarray(inputs["out_b"], dtype=np.float32)
    out = np.empty((2, S, DM), np.float32)
    for b in range(2):
        for tc_i in range(2):
            part = (np.asarray(res.results[_core_index(b, tc_i, 0)]["outT"], dtype=np.float32)
                    + np.asarray(res.results[_core_index(b, tc_i, 1)]["outT"], dtype=np.float32))
            out[b, tc_i * T:(tc_i + 1) * T, :] = part + out_b[None, :]
    return out, res


def kernel(**inputs):
    out, _ = run(inputs, trace=False)
    return out
